# revision 1
# baseline (speedup 1.0000x reference)
"""CrossAttention Trainium2 kernel (8-core SPMD, tensor-parallel over (batch, head-pair)).

Reference computation (full):
    q = x @ Wq; k = ctx @ Wk; v = ctx @ Wv            (per-head split, D=64)
    attn = softmax(q k^T / sqrt(D)) @ v
    out = attn @ Wo + bo

Sharding: core c in [0,8) handles batch b = c // 4 and head-pair hp = c % 4
(heads 2*hp, 2*hp+1 -> 128 "inner" dims, a full PE-array width). Each core
produces a partial output [Sq, 512] (its two heads' contribution through Wo);
the host sums the 4 partials per batch and adds the bias.

On-device layout (per core), everything bf16 except PSUM accum / softmax sums:
  QT [128, 4096] = (x @ Wq_2h)^T      via lhsT=Wq tiles, rhs=xT (host-transposed)
  KT [128, 4096] = (ctx @ Wk_2h)^T
  V  [s, 128]    = ctx @ Wv_2h        (natural orientation, per s-tile)
  ST[s,q] scores computed transposed (softmax sums via ones-matmul), heads
  row-packed in the PE array (K=64 each); exp on ScalarE straight from PSUM
  with the 1/8 scale folded in (exp is the roofline engine: ~33.6M elem/core).
  attnT[d2h, q] accumulated over s-tiles with heads col-packed; normalized by
  1/sums (DVE reciprocal, replicated across partitions via a DRAM-bounce
  broadcast DMA); out tile = attnT_scaled^T-matmul with Wo_2h.
"""

import sys

sys.path.insert(0, "/opt/trn_rl_repo")

import numpy as np
import ml_dtypes

BF16 = ml_dtypes.bfloat16

B, SQ, DM = 2, 4096, 512
SKV, DC = 4096, 768
H, D = 8, 64
INNER = H * D  # 512
D2H = 2 * D  # 128, inner dims per core
N_CORES = 8
P = 128
QCHUNK = 512
N_QCHUNK = SQ // QCHUNK  # 8
N_STILE = SKV // P  # 32
KT_Q = DM // P  # 4 k-tiles for the Q projection
KT_KV = DC // P  # 6 k-tiles for the K/V projections
SCALE = float(D) ** -0.5

_COMPILED = None


def _build():
    import concourse.bass as bass
    import concourse.tile as tile
    from concourse import bacc, mybir

    fp32 = mybir.dt.float32
    bf16 = mybir.dt.bfloat16
    Exp = mybir.ActivationFunctionType.Exp

    nc = bacc.Bacc(
        "TRN2",
        target_bir_lowering=False,
        debug=False,
        enable_asserts=False,
        num_devices=N_CORES,
    )

    xT = nc.dram_tensor("xT", [DM, SQ], bf16, kind="ExternalInput").ap()
    ctxT = nc.dram_tensor("ctxT", [DC, SKV], bf16, kind="ExternalInput").ap()
    wq = nc.dram_tensor("wq", [DM, D2H], bf16, kind="ExternalInput").ap()
    wk = nc.dram_tensor("wk", [DC, D2H], bf16, kind="ExternalInput").ap()
    wv = nc.dram_tensor("wv", [DC, D2H], bf16, kind="ExternalInput").ap()
    wo = nc.dram_tensor("wo", [D2H, INNER], bf16, kind="ExternalInput").ap()
    out = nc.dram_tensor("out", [SQ, INNER], fp32, kind="ExternalOutput").ap()

    with tile.TileContext(nc) as tc:
        with (
            tc.tile_pool(name="persist", bufs=1) as persist,
            tc.tile_pool(name="pp", bufs=2, space="PSUM") as pp,
            tc.tile_pool(name="spsum", bufs=2, space="PSUM") as spsum,
            tc.tile_pool(name="atpsum", bufs=1, space="PSUM") as atpsum,
            tc.tile_pool(name="smpsum", bufs=1, space="PSUM") as smpsum,
            tc.tile_pool(name="epool", bufs=6) as epool,
            tc.tile_pool(name="npool", bufs=2) as npool,
            tc.tile_pool(name="opool", bufs=3) as opool,
            tc.tile_pool(name="dscr", bufs=2, space="DRAM") as dscr,
        ):
            # --- persistent SBUF tensors
            xT_sb = persist.tile([P, KT_Q, SQ], bf16)
            ctxT_sb = persist.tile([P, KT_KV, SKV], bf16)
            wq_sb = persist.tile([P, KT_Q, D2H], bf16)
            wk_sb = persist.tile([P, KT_KV, D2H], bf16)
            wv_sb = persist.tile([P, KT_KV, D2H], bf16)
            wo_sb = persist.tile([P, INNER], bf16)
            qt_sb = persist.tile([P, SQ], bf16)
            kt_sb = persist.tile([P, SQ], bf16)
            v_sb = persist.tile([P, N_STILE * D2H], bf16)
            asc_sb = persist.tile([P, SQ], bf16)  # normalized attnT
            ones_sb = persist.tile([P, 1], bf16)
            junk_sb = persist.tile([P, 8], fp32)

            nc.vector.memset(ones_sb, 1.0)
            # Preload the exp table set during the DMA phase.
            nc.vector.memset(junk_sb, 0.0)
            nc.scalar.activation(out=junk_sb, in_=junk_sb, func=Exp)

            # --- input DMAs
            nc.sync.dma_start(out=wq_sb, in_=wq.rearrange("(t p) m -> p t m", p=P))
            nc.sync.dma_start(out=wk_sb, in_=wk.rearrange("(t p) m -> p t m", p=P))
            nc.sync.dma_start(out=wv_sb, in_=wv.rearrange("(t p) m -> p t m", p=P))
            nc.sync.dma_start(out=wo_sb, in_=wo)
            for t in range(KT_Q):
                nc.sync.dma_start(out=xT_sb[:, t, :], in_=xT[t * P : (t + 1) * P, :])
            for t in range(KT_KV):
                nc.sync.dma_start(
                    out=ctxT_sb[:, t, :], in_=ctxT[t * P : (t + 1) * P, :]
                )

            # --- projections, interleaved with chunk-0 attention so the
            # ScalarE exp stream (the bottleneck engine) starts ~3.5us in
            # instead of after all projections.
            def emit_qt(c):
                cs = slice(c * QCHUNK, (c + 1) * QCHUNK)
                ps = pp.tile([P, QCHUNK], fp32, tag="pp", name=f"qtp{c}")
                for t in range(KT_Q):
                    nc.tensor.matmul(
                        out=ps,
                        lhsT=wq_sb[:, t, :],
                        rhs=xT_sb[:, t, cs],
                        start=(t == 0),
                        stop=(t == KT_Q - 1),
                    )
                nc.vector.tensor_copy(out=qt_sb[:, cs], in_=ps)

            def emit_kt(c):
                cs = slice(c * QCHUNK, (c + 1) * QCHUNK)
                ps = pp.tile([P, QCHUNK], fp32, tag="pp", name=f"ktp{c}")
                for t in range(KT_KV):
                    nc.tensor.matmul(
                        out=ps,
                        lhsT=wk_sb[:, t, :],
                        rhs=ctxT_sb[:, t, cs],
                        start=(t == 0),
                        stop=(t == KT_KV - 1),
                    )
                nc.vector.tensor_copy(out=kt_sb[:, cs], in_=ps)

            def emit_v(si):
                ss = slice(si * P, (si + 1) * P)
                ps = pp.tile([P, D2H], fp32, tag="pp", name=f"vp{si}")
                for t in range(KT_KV):
                    nc.tensor.matmul(
                        out=ps,
                        lhsT=ctxT_sb[:, t, ss],
                        rhs=wv_sb[:, t, :],
                        start=(t == 0),
                        stop=(t == KT_KV - 1),
                    )
                nc.vector.tensor_copy(
                    out=v_sb[:, si * D2H : (si + 1) * D2H], in_=ps
                )

            def emit_attn_step(c, si, at_ps, sm_ps):
                cs = slice(c * QCHUNK, (c + 1) * QCHUNK)
                ss = slice(si * P, (si + 1) * P)
                sp = spsum.tile([P, 2 * QCHUNK], fp32, tag="sp", name=f"sp{c}_{si}")
                # scores^T, heads row-packed (K=64 each)
                nc.tensor.matmul(
                    out=sp[:, 0:QCHUNK],
                    lhsT=kt_sb[0:64, ss],
                    rhs=qt_sb[0:64, cs],
                    start=True,
                    stop=True,
                )
                nc.tensor.matmul(
                    out=sp[:, QCHUNK : 2 * QCHUNK],
                    lhsT=kt_sb[64:128, ss],
                    rhs=qt_sb[64:128, cs],
                    start=True,
                    stop=True,
                )
                es = epool.tile([P, 2 * QCHUNK], bf16, tag="es", name=f"es{c}_{si}")
                nc.scalar.activation(out=es, in_=sp, func=Exp, scale=SCALE)
                # attnT accumulation, heads col-packed
                vs = si * D2H
                nc.tensor.matmul(
                    out=at_ps[0:64, :],
                    lhsT=v_sb[:, vs : vs + 64],
                    rhs=es[:, 0:QCHUNK],
                    start=(si == 0),
                    stop=(si == N_STILE - 1),
                )
                nc.tensor.matmul(
                    out=at_ps[64:128, :],
                    lhsT=v_sb[:, vs + 64 : vs + 128],
                    rhs=es[:, QCHUNK : 2 * QCHUNK],
                    start=(si == 0),
                    stop=(si == N_STILE - 1),
                )
                # softmax denominators, col-packed at partitions 0 / 32
                nc.tensor.matmul(
                    out=sm_ps[0:1, :],
                    lhsT=ones_sb,
                    rhs=es[:, 0:QCHUNK],
                    start=(si == 0),
                    stop=(si == N_STILE - 1),
                )
                nc.tensor.matmul(
                    out=sm_ps[32:33, :],
                    lhsT=ones_sb,
                    rhs=es[:, QCHUNK : 2 * QCHUNK],
                    start=(si == 0),
                    stop=(si == N_STILE - 1),
                )

            # chunk 0 runs interleaved with KT/V production (its s-tile si
            # only needs KT cols/V tile si); remaining QT chunks slot in
            # between s-tile groups.
            emit_qt(0)
            at_ps0 = atpsum.tile([P, QCHUNK], fp32, tag="at", name="at0")
            sm_ps0 = smpsum.tile([P, QCHUNK], fp32, tag="sm", name="sm0")
            for ck in range(N_QCHUNK):
                emit_kt(ck)
                for si in range(4 * ck, 4 * ck + 4):
                    emit_v(si)
                    emit_attn_step(0, si, at_ps0, sm_ps0)
                if ck < N_QCHUNK - 1:
                    emit_qt(ck + 1)

            # --- attention for remaining q-chunks
            for c in range(N_QCHUNK):
                cs = slice(c * QCHUNK, (c + 1) * QCHUNK)
                if c == 0:
                    at_ps, sm_ps = at_ps0, sm_ps0
                else:
                    at_ps = atpsum.tile([P, QCHUNK], fp32, tag="at", name=f"at{c}")
                    sm_ps = smpsum.tile([P, QCHUNK], fp32, tag="sm", name=f"sm{c}")
                    for si in range(N_STILE):
                        emit_attn_step(c, si, at_ps, sm_ps)

                # normalization: recip of sums, broadcast across partitions
                rec_sb = npool.tile([P, QCHUNK], fp32, tag="rec")
                nc.vector.reciprocal(out=rec_sb[0:1, :], in_=sm_ps[0:1, :])
                nc.vector.reciprocal(out=rec_sb[32:33, :], in_=sm_ps[32:33, :])
                dt = dscr.tile([2, QCHUNK], fp32, tag="dt")
                nc.sync.dma_start(out=dt[0:1, :], in_=rec_sb[0:1, :])
                nc.sync.dma_start(out=dt[1:2, :], in_=rec_sb[32:33, :])
                rep_sb = npool.tile([P, QCHUNK], fp32, tag="rep")
                import concourse.bass as _b

                bcast0 = _b.AP(
                    tensor=dt.tensor, offset=dt.offset, ap=[[0, 64], [1, QCHUNK]]
                )
                bcast1 = _b.AP(
                    tensor=dt.tensor,
                    offset=dt.offset + QCHUNK,
                    ap=[[0, 64], [1, QCHUNK]],
                )
                nc.sync.dma_start(out=rep_sb[0:64, :], in_=bcast0)
                nc.sync.dma_start(out=rep_sb[64:128, :], in_=bcast1)
                nc.vector.tensor_mul(asc_sb[:, cs], at_ps, rep_sb)

                # output projection for this chunk's q-tiles
                for qt in range(QCHUNK // P):
                    r0 = c * QCHUNK + qt * P
                    po = pp.tile([P, INNER], fp32, tag="pp")
                    nc.tensor.matmul(
                        out=po,
                        lhsT=asc_sb[:, r0 : r0 + P],
                        rhs=wo_sb,
                        start=True,
                        stop=True,
                    )
                    ob = opool.tile([P, INNER], fp32, tag="ob")
                    nc.vector.tensor_copy(out=ob, in_=po)
                    nc.sync.dma_start(out=out[r0 : r0 + P, :], in_=ob)

    nc.compile()
    return nc


def _get_compiled():
    global _COMPILED
    if _COMPILED is None:
        _COMPILED = _build()
    return _COMPILED


def _make_in_maps(x, context, Wq, Wk, Wv, Wo):
    xT = [np.ascontiguousarray(x[b].T).astype(BF16) for b in range(B)]
    ctxT = [np.ascontiguousarray(context[b].T).astype(BF16) for b in range(B)]
    wq16, wk16 = Wq.astype(BF16), Wk.astype(BF16)
    wv16, wo16 = Wv.astype(BF16), Wo.astype(BF16)
    in_maps = []
    for core in range(N_CORES):
        b, hp = core // 4, core % 4
        js = slice(hp * D2H, (hp + 1) * D2H)
        in_maps.append(
            {
                "xT": xT[b],
                "ctxT": ctxT[b],
                "wq": np.ascontiguousarray(wq16[:, js]),
                "wk": np.ascontiguousarray(wk16[:, js]),
                "wv": np.ascontiguousarray(wv16[:, js]),
                "wo": np.ascontiguousarray(wo16[js, :]),
            }
        )
    return in_maps


def run(inputs, **kw):
    """Run on hardware; returns (full_output, results list)."""
    from concourse import bass2jax

    nc = _get_compiled()
    in_maps = _make_in_maps(
        inputs["x"], inputs["context"], inputs["Wq"], inputs["Wk"],
        inputs["Wv"], inputs["Wo"],
    )
    results = bass2jax.run_bass_via_pjrt(nc, in_maps, n_cores=N_CORES)
    bo = inputs["bo"]
    out = np.empty((B, SQ, INNER), np.float32)
    for b in range(B):
        acc = results[4 * b]["out"].astype(np.float32)
        for hp in range(1, 4):
            acc = acc + results[4 * b + hp]["out"]
        out[b] = acc + np.asarray(bo, np.float32)[None, :]
    return out, results


def time_exec(inputs, iters=256):
    nc = _get_compiled()
    in_maps = _make_in_maps(
        inputs["x"], inputs["context"], inputs["Wq"], inputs["Wk"],
        inputs["Wv"], inputs["Wo"],
    )
    return time_nc(nc, in_maps, iters=iters)


def time_nc(nc, in_maps, iters=256):
    """Amortized device execution time per kernel launch, in ns.

    Replicates bass2jax.run_bass_via_pjrt's multi-core shard_map body, stages
    inputs + donated (device-created) zero output buffers, then dispatches
    `iters` executions asynchronously; reports the marginal time per call
    between a short and a long batch to cancel fixed dispatch overheads.
    """
    import time as _time

    import jax
    from jax.sharding import Mesh, NamedSharding, PartitionSpec
    from concourse import bass2jax, mybir
    from concourse.bass2jax import _bass_exec_p, install_neuronx_cc_hook

    try:
        from jax.experimental.shard_map import shard_map
    except ImportError:
        from jax.shard_map import shard_map

    install_neuronx_cc_hook()

    partition_name = nc.partition_id_tensor.name if nc.partition_id_tensor else None
    in_names, out_names, out_avals, zero_outs = [], [], [], []
    for alloc in nc.m.functions[0].allocations:
        if not isinstance(alloc, mybir.MemoryLocationSet):
            continue
        name = alloc.memorylocations[0].name
        if alloc.kind == "ExternalInput":
            if name != partition_name:
                in_names.append(name)
        elif alloc.kind == "ExternalOutput":
            out_names.append(name)
            shape = tuple(alloc.tensor_shape)
            dtype = mybir.dt.np(alloc.dtype)
            out_avals.append(jax.core.ShapedArray(shape, dtype))
            zero_outs.append(np.zeros(shape, dtype))
    n_params = len(in_names)
    n_outs = len(out_avals)
    in_names = in_names + out_names
    if partition_name is not None:
        in_names.append(partition_name)
    donate = tuple(range(n_params, n_params + n_outs))

    def _body(*args):
        operands = list(args)
        if partition_name is not None:
            operands.append(bass2jax.partition_id_tensor())
        outs = _bass_exec_p.bind(
            *operands,
            out_avals=tuple(out_avals),
            in_names=tuple(in_names),
            out_names=tuple(out_names),
            lowering_input_output_aliases=(),
            sim_require_finite=True,
            sim_require_nnan=True,
            nc=nc,
        )
        return tuple(outs)

    devices = jax.devices()[:N_CORES]
    mesh = Mesh(np.asarray(devices), ("core",))
    in_specs = (PartitionSpec("core"),) * (n_params + n_outs)
    out_specs = (PartitionSpec("core"),) * n_outs
    sharded = jax.jit(
        shard_map(
            _body, mesh=mesh, in_specs=in_specs, out_specs=out_specs, check_rep=False
        ),
        donate_argnums=donate,
        keep_unused=True,
    )
    sh = NamedSharding(mesh, PartitionSpec("core"))
    concat_in = [
        jax.device_put(
            np.concatenate(
                [np.asarray(in_maps[c][in_names[i]]) for c in range(N_CORES)], axis=0
            ),
            sh,
        )
        for i in range(n_params)
    ]
    import jax.numpy as jnp

    zshapes = [((N_CORES * z.shape[0], *z.shape[1:]), z.dtype) for z in zero_outs]
    mkzeros = jax.jit(
        lambda: tuple(jnp.zeros(s, d) for s, d in zshapes),
        out_shardings=tuple(sh for _ in zshapes),
    )
    # warmup + compile
    out = sharded(*concat_in, *mkzeros())
    jax.block_until_ready(out)

    def measure(n):
        zs = [mkzeros() for _ in range(n)]
        jax.block_until_ready(zs)
        jax.block_until_ready(concat_in)
        outs = []
        t0 = _time.perf_counter()
        for k in range(n):
            outs.append(sharded(*concat_in, *zs[k]))
        jax.block_until_ready(outs)
        return _time.perf_counter() - t0

    measure(4)  # warm the dispatch path
    lo, hi = max(8, iters // 4), iters
    t_lo, t_hi = measure(lo), measure(hi)
    marginal = (t_hi - t_lo) / (hi - lo) * 1e9
    per_call = t_hi / hi * 1e9
    print(f"  [time_nc] lo={lo}:{t_lo * 1e3:.1f}ms hi={hi}:{t_hi * 1e3:.1f}ms "
          f"marginal={marginal / 1e3:.1f}us percall={per_call / 1e3:.1f}us")
    return marginal if marginal > 0 else per_call


def kernel(**inputs) -> np.ndarray:
    out, _ = run(inputs)
    return out



# revision 7
# speedup vs baseline: 1.0922x; 1.0922x over previous
"""CrossAttention Trainium2 kernel (8-core SPMD, tensor-parallel over (batch, head-pair)).

Reference computation (full):
    q = x @ Wq; k = ctx @ Wk; v = ctx @ Wv            (per-head split, D=64)
    attn = softmax(q k^T / sqrt(D)) @ v
    out = attn @ Wo + bo

Sharding: core c in [0,8) handles batch b = c // 4 and head-pair hp = c % 4
(heads 2*hp, 2*hp+1 -> 128 "inner" dims). Each core produces a partial output
[Sq, 512] (its two heads' contribution through Wo); the host sums the 4
partials per batch and adds the bias.

Device schedule (per core), built to keep the ScalarE exp stream (the roofline
engine: 33.5M exp elems/core ~ 255us) saturated:
  - scores computed transposed, the two heads' K=64 matmuls packed in PE row
    groups 0/64 (concurrent); attnT accumulation col-packed at partitions
    0-63/64-127 (concurrent); softmax-denominator ones-matmuls col-packed at
    partitions 0/32 (concurrent) -> ~1536 PE cycles per (stile, qchunk) step
    vs 1024-cycle exp.
  - inputs DMA'd in 512-col chunks so the first kt/v tiles (and hence exp)
    start ~5us in; qt for chunk c+1 projected mid-chunk c.
  - chunk boundaries decoupled: at_ps is copied to SBUF right after its last
    accumulation (frees the single PSUM accumulator); denominators alternate
    PSUM partitions 0/32 vs 64/96 between chunks; the reciprocal ->
    DRAM-bounce broadcast -> normalize -> Wo projection chain runs off the
    critical path.
  PSUM: sp 2x[128,1024] (4 banks) + at [128,512] (1) + sm (1) + pp 2x (2).

`_build(repeat=R)` emits the whole body R times in one NEFF; timing two
repeat counts and differencing cancels the ~0.9ms/launch axon dispatch
overhead, isolating true device time per iteration.
"""

import sys

sys.path.insert(0, "/opt/trn_rl_repo")

import numpy as np
import ml_dtypes

BF16 = ml_dtypes.bfloat16

B, SQ, DM = 2, 4096, 512
SKV, DC = 4096, 768
H, D = 8, 64
INNER = H * D  # 512
D2H = 2 * D  # 128, inner dims per core
N_CORES = 8
P = 128
QCHUNK = 512
N_QCHUNK = SQ // QCHUNK  # 8
N_STILE = SKV // P  # 32
KT_Q = DM // P  # 4 k-tiles for the Q projection
KT_KV = DC // P  # 6 k-tiles for the K/V projections
SCALE = float(D) ** -0.5

_COMPILED = None
_COMPILED_R = {}


def _build(repeat=1):
    import concourse.bass as bass
    import concourse.tile as tile
    from concourse import bacc, mybir

    fp32 = mybir.dt.float32
    bf16 = mybir.dt.bfloat16
    Exp = mybir.ActivationFunctionType.Exp

    nc = bacc.Bacc(
        "TRN2",
        target_bir_lowering=False,
        debug=False,
        enable_asserts=False,
        num_devices=N_CORES,
    )

    xT = nc.dram_tensor("xT", [DM, SQ], bf16, kind="ExternalInput").ap()
    ctxT = nc.dram_tensor("ctxT", [DC, SKV], bf16, kind="ExternalInput").ap()
    wq = nc.dram_tensor("wq", [DM, D2H], bf16, kind="ExternalInput").ap()
    wk = nc.dram_tensor("wk", [DC, D2H], bf16, kind="ExternalInput").ap()
    wv = nc.dram_tensor("wv", [DC, D2H], bf16, kind="ExternalInput").ap()
    wo = nc.dram_tensor("wo", [D2H, INNER], bf16, kind="ExternalInput").ap()
    out = nc.dram_tensor("out", [SQ, INNER], fp32, kind="ExternalOutput").ap()

    with tile.TileContext(nc) as tc:
        with (
            tc.tile_pool(name="persist", bufs=1) as persist,
            tc.tile_pool(name="pp", bufs=2, space="PSUM") as pp,
            tc.tile_pool(name="spsum", bufs=2, space="PSUM") as spsum,
            tc.tile_pool(name="atpsum", bufs=1, space="PSUM") as atpsum,
            tc.tile_pool(name="smpsum", bufs=1, space="PSUM") as smpsum,
            tc.tile_pool(name="epool", bufs=6) as epool,
            tc.tile_pool(name="apool", bufs=2) as apool,
            tc.tile_pool(name="npool", bufs=2) as npool,
            tc.tile_pool(name="opool", bufs=3) as opool,
            tc.tile_pool(name="dscr", bufs=2, space="DRAM") as dscr,
        ):
            # --- persistent SBUF tensors
            xT_sb = persist.tile([P, KT_Q, SQ], bf16)
            ctxT_sb = persist.tile([P, KT_KV, SKV], bf16)
            wq_sb = persist.tile([P, KT_Q, D2H], bf16)
            wk_sb = persist.tile([P, KT_KV, D2H], bf16)
            wv_sb = persist.tile([P, KT_KV, D2H], bf16)
            wo_sb = persist.tile([P, INNER], bf16)
            qt_sb = persist.tile([P, SQ], bf16)
            kt_sb = persist.tile([P, SQ], bf16)
            v_sb = persist.tile([P, N_STILE * D2H], bf16)
            asc_sb = persist.tile([P, SQ], bf16)  # normalized attnT
            ones_sb = persist.tile([P, 1], bf16)
            junk_sb = persist.tile([P, 8], fp32)

            nc.vector.memset(ones_sb, 1.0)
            # Preload the exp table set during the DMA phase.
            nc.vector.memset(junk_sb, 0.0)
            nc.scalar.activation(out=junk_sb, in_=junk_sb, func=Exp)

            def emit_rep(rep):
                r = f"r{rep}_"

                # --- input DMAs, chunked 512 cols so first tiles land early
                def dma_x_chunk(c):
                    cs = slice(c * QCHUNK, (c + 1) * QCHUNK)
                    for t in range(KT_Q):
                        nc.sync.dma_start(
                            out=xT_sb[:, t, cs], in_=xT[t * P : (t + 1) * P, cs]
                        )

                def dma_ctx_chunk(c):
                    cs = slice(c * QCHUNK, (c + 1) * QCHUNK)
                    for t in range(KT_KV):
                        nc.sync.dma_start(
                            out=ctxT_sb[:, t, cs], in_=ctxT[t * P : (t + 1) * P, cs]
                        )

                nc.sync.dma_start(
                    out=wq_sb, in_=wq.rearrange("(t p) m -> p t m", p=P)
                )
                dma_x_chunk(0)
                nc.sync.dma_start(
                    out=wk_sb, in_=wk.rearrange("(t p) m -> p t m", p=P)
                )
                nc.sync.dma_start(
                    out=wv_sb, in_=wv.rearrange("(t p) m -> p t m", p=P)
                )
                dma_ctx_chunk(0)
                nc.sync.dma_start(out=wo_sb, in_=wo)
                for c in range(1, N_QCHUNK):
                    dma_x_chunk(c)
                    dma_ctx_chunk(c)

                # --- projections
                def emit_qt(c):
                    cs = slice(c * QCHUNK, (c + 1) * QCHUNK)
                    ps = pp.tile([P, QCHUNK], fp32, tag="pp", name=f"{r}qtp{c}")
                    for t in range(KT_Q):
                        nc.tensor.matmul(
                            out=ps,
                            lhsT=wq_sb[:, t, :],
                            rhs=xT_sb[:, t, cs],
                            start=(t == 0),
                            stop=(t == KT_Q - 1),
                        )
                    nc.vector.tensor_copy(out=qt_sb[:, cs], in_=ps)

                def emit_kt(c):
                    cs = slice(c * QCHUNK, (c + 1) * QCHUNK)
                    ps = pp.tile([P, QCHUNK], fp32, tag="pp", name=f"{r}ktp{c}")
                    for t in range(KT_KV):
                        nc.tensor.matmul(
                            out=ps,
                            lhsT=wk_sb[:, t, :],
                            rhs=ctxT_sb[:, t, cs],
                            start=(t == 0),
                            stop=(t == KT_KV - 1),
                        )
                    nc.vector.tensor_copy(out=kt_sb[:, cs], in_=ps)

                def emit_v(si):
                    ss = slice(si * P, (si + 1) * P)
                    ps = pp.tile([P, D2H], fp32, tag="pp", name=f"{r}vp{si}")
                    for t in range(KT_KV):
                        nc.tensor.matmul(
                            out=ps,
                            lhsT=ctxT_sb[:, t, ss],
                            rhs=wv_sb[:, t, :],
                            start=(t == 0),
                            stop=(t == KT_KV - 1),
                        )
                    nc.vector.tensor_copy(
                        out=v_sb[:, si * D2H : (si + 1) * D2H], in_=ps
                    )

                def emit_attn_step(c, si, at_ps, sm_ps, p0):
                    cs = slice(c * QCHUNK, (c + 1) * QCHUNK)
                    ss = slice(si * P, (si + 1) * P)
                    sp = spsum.tile(
                        [P, 2 * QCHUNK], fp32, tag="sp", name=f"{r}sp{c}_{si}"
                    )
                    # scores^T, heads row-packed (K=64, PE rows 0/64: concurrent)
                    nc.tensor.matmul(
                        out=sp[:, 0:QCHUNK],
                        lhsT=kt_sb[0:64, ss],
                        rhs=qt_sb[0:64, cs],
                        start=True,
                        stop=True,
                    )
                    nc.tensor.matmul(
                        out=sp[:, QCHUNK : 2 * QCHUNK],
                        lhsT=kt_sb[64:128, ss],
                        rhs=qt_sb[64:128, cs],
                        start=True,
                        stop=True,
                    )
                    es = epool.tile(
                        [P, 2 * QCHUNK], bf16, tag="es", name=f"{r}es{c}_{si}"
                    )
                    nc.scalar.activation(out=es, in_=sp, func=Exp, scale=SCALE)
                    # attnT accumulation, heads col-packed (PE cols 0/64)
                    vs = si * D2H
                    nc.tensor.matmul(
                        out=at_ps[0:64, :],
                        lhsT=v_sb[:, vs : vs + 64],
                        rhs=es[:, 0:QCHUNK],
                        start=(si == 0),
                        stop=(si == N_STILE - 1),
                    )
                    nc.tensor.matmul(
                        out=at_ps[64:128, :],
                        lhsT=v_sb[:, vs + 64 : vs + 128],
                        rhs=es[:, QCHUNK : 2 * QCHUNK],
                        start=(si == 0),
                        stop=(si == N_STILE - 1),
                        # sim's psum group-tracking is partition-blind; HW
                        # tracks per-element has_written, so col-packed
                        # groups are safe
                        skip_group_check=True,
                    )
                    # softmax denominators, col-packed at partitions p0 / p0+32
                    # (p0 alternates 0/64 between chunks: no inter-chunk hazard)
                    nc.tensor.matmul(
                        out=sm_ps[p0 : p0 + 1, :],
                        lhsT=ones_sb,
                        rhs=es[:, 0:QCHUNK],
                        start=(si == 0),
                        stop=(si == N_STILE - 1),
                        tile_position=(0, p0),
                    )
                    nc.tensor.matmul(
                        out=sm_ps[p0 + 32 : p0 + 33, :],
                        lhsT=ones_sb,
                        rhs=es[:, QCHUNK : 2 * QCHUNK],
                        start=(si == 0),
                        stop=(si == N_STILE - 1),
                        tile_position=(0, p0 + 32),
                        skip_group_check=True,
                    )

                def finish_chunk(c, at_ps, sm_ps, p0):
                    """Free the accumulators, then normalize + project."""
                    cs = slice(c * QCHUNK, (c + 1) * QCHUNK)
                    araw = apool.tile(
                        [P, QCHUNK], bf16, tag="araw", name=f"{r}ar{c}"
                    )
                    nc.vector.tensor_copy(out=araw, in_=at_ps)
                    rec_sb = npool.tile(
                        [P, QCHUNK], fp32, tag="rec", name=f"{r}rc{c}"
                    )
                    nc.vector.reciprocal(
                        out=rec_sb[0:1, :], in_=sm_ps[p0 : p0 + 1, :]
                    )
                    nc.vector.reciprocal(
                        out=rec_sb[32:33, :], in_=sm_ps[p0 + 32 : p0 + 33, :]
                    )
                    dt = dscr.tile([2, QCHUNK], fp32, tag="dt", name=f"{r}dt{c}")
                    nc.sync.dma_start(out=dt[0:1, :], in_=rec_sb[0:1, :])
                    nc.sync.dma_start(out=dt[1:2, :], in_=rec_sb[32:33, :])
                    rep_sb = npool.tile(
                        [P, QCHUNK], fp32, tag="rep", name=f"{r}rp{c}"
                    )
                    bcast0 = bass.AP(
                        tensor=dt.tensor, offset=dt.offset, ap=[[0, 64], [1, QCHUNK]]
                    )
                    bcast1 = bass.AP(
                        tensor=dt.tensor,
                        offset=dt.offset + QCHUNK,
                        ap=[[0, 64], [1, QCHUNK]],
                    )
                    nc.sync.dma_start(out=rep_sb[0:64, :], in_=bcast0)
                    nc.sync.dma_start(out=rep_sb[64:128, :], in_=bcast1)
                    nc.vector.tensor_mul(asc_sb[:, cs], araw, rep_sb)
                    for qt in range(QCHUNK // P):
                        r0 = c * QCHUNK + qt * P
                        po = pp.tile(
                            [P, INNER], fp32, tag="pp", name=f"{r}po{c}_{qt}"
                        )
                        nc.tensor.matmul(
                            out=po,
                            lhsT=asc_sb[:, r0 : r0 + P],
                            rhs=wo_sb,
                            start=True,
                            stop=True,
                        )
                        ob = opool.tile(
                            [P, INNER], fp32, tag="ob", name=f"{r}ob{c}_{qt}"
                        )
                        nc.vector.tensor_copy(out=ob, in_=po)
                        nc.sync.dma_start(out=out[r0 : r0 + P, :], in_=ob)

                # --- phase A: chunk-0 attention interleaved with kt/v production
                emit_qt(0)
                at_ps = atpsum.tile([P, QCHUNK], fp32, tag="at", name=f"{r}at0")
                sm_ps = smpsum.tile([P, QCHUNK], fp32, tag="sm", name=f"{r}sm0")
                for ck in range(N_QCHUNK):
                    emit_kt(ck)
                    for si in range(4 * ck, 4 * ck + 4):
                        emit_v(si)
                        emit_attn_step(0, si, at_ps, sm_ps, 0)
                emit_qt(1)
                finish_chunk(0, at_ps, sm_ps, 0)

                # --- remaining q-chunks; qt for the next chunk is projected
                # mid-chunk so its copy lands before the chunk boundary.
                for c in range(1, N_QCHUNK):
                    p0 = 64 * (c % 2)
                    at_ps = atpsum.tile(
                        [P, QCHUNK], fp32, tag="at", name=f"{r}at{c}"
                    )
                    sm_ps = smpsum.tile(
                        [P, QCHUNK], fp32, tag="sm", name=f"{r}sm{c}"
                    )
                    for si in range(N_STILE):
                        emit_attn_step(c, si, at_ps, sm_ps, p0)
                        if si == 16 and c < N_QCHUNK - 1:
                            emit_qt(c + 1)
                    finish_chunk(c, at_ps, sm_ps, p0)

            for rep in range(repeat):
                emit_rep(rep)

    nc.compile()
    return nc


def _get_compiled():
    global _COMPILED
    if _COMPILED is None:
        _COMPILED = _build()
    return _COMPILED


def _get_compiled_r(repeat):
    if repeat == 1:
        return _get_compiled()
    if repeat not in _COMPILED_R:
        _COMPILED_R[repeat] = _build(repeat)
    return _COMPILED_R[repeat]


def _make_in_maps(x, context, Wq, Wk, Wv, Wo):
    xT = [np.ascontiguousarray(x[b].T).astype(BF16) for b in range(B)]
    ctxT = [np.ascontiguousarray(context[b].T).astype(BF16) for b in range(B)]
    wq16, wk16 = Wq.astype(BF16), Wk.astype(BF16)
    wv16, wo16 = Wv.astype(BF16), Wo.astype(BF16)
    in_maps = []
    for core in range(N_CORES):
        b, hp = core // 4, core % 4
        js = slice(hp * D2H, (hp + 1) * D2H)
        in_maps.append(
            {
                "xT": xT[b],
                "ctxT": ctxT[b],
                "wq": np.ascontiguousarray(wq16[:, js]),
                "wk": np.ascontiguousarray(wk16[:, js]),
                "wv": np.ascontiguousarray(wv16[:, js]),
                "wo": np.ascontiguousarray(wo16[js, :]),
            }
        )
    return in_maps


def run(inputs, **kw):
    """Run on hardware; returns (full_output, results list)."""
    from concourse import bass2jax

    nc = _get_compiled()
    in_maps = _make_in_maps(
        inputs["x"], inputs["context"], inputs["Wq"], inputs["Wk"],
        inputs["Wv"], inputs["Wo"],
    )
    results = bass2jax.run_bass_via_pjrt(nc, in_maps, n_cores=N_CORES)
    bo = inputs["bo"]
    out = np.empty((B, SQ, INNER), np.float32)
    for b in range(B):
        acc = results[4 * b]["out"].astype(np.float32)
        for hp in range(1, 4):
            acc = acc + results[4 * b + hp]["out"]
        out[b] = acc + np.asarray(bo, np.float32)[None, :]
    return out, results


def time_exec(inputs, iters=48):
    """True device time per kernel iteration, in ns.

    Per-XLA-call dispatch through the axon tunnel costs ~0.7-0.9ms and masks
    device time entirely (a trivial kernel's per-call marginal measures the
    same as this kernel's). So we compile two NEFFs that run the body R=4 and
    R=8 times and difference the per-call marginals: (m8 - m4) / 4 cancels
    the dispatch constant exactly, leaving pure device execution time.
    """
    in_maps = _make_in_maps(
        inputs["x"], inputs["context"], inputs["Wq"], inputs["Wk"],
        inputs["Wv"], inputs["Wo"],
    )
    m4 = time_nc(_get_compiled_r(4), in_maps, iters=iters)
    m8 = time_nc(_get_compiled_r(8), in_maps, iters=iters)
    exec_ns = (m8 - m4) / 4.0
    print(f"  [time_exec] m4={m4 / 1e3:.1f}us m8={m8 / 1e3:.1f}us "
          f"-> exec {exec_ns / 1e3:.1f}us")
    return exec_ns


def time_nc(nc, in_maps, iters=48):
    """Amortized wall time per XLA call (includes dispatch overhead), in ns."""
    import time as _time

    import jax
    from jax.sharding import Mesh, NamedSharding, PartitionSpec
    from concourse import bass2jax, mybir
    from concourse.bass2jax import _bass_exec_p, install_neuronx_cc_hook

    try:
        from jax.experimental.shard_map import shard_map
    except ImportError:
        from jax.shard_map import shard_map

    install_neuronx_cc_hook()

    partition_name = nc.partition_id_tensor.name if nc.partition_id_tensor else None
    in_names, out_names, out_avals, zero_outs = [], [], [], []
    for alloc in nc.m.functions[0].allocations:
        if not isinstance(alloc, mybir.MemoryLocationSet):
            continue
        name = alloc.memorylocations[0].name
        if alloc.kind == "ExternalInput":
            if name != partition_name:
                in_names.append(name)
        elif alloc.kind == "ExternalOutput":
            out_names.append(name)
            shape = tuple(alloc.tensor_shape)
            dtype = mybir.dt.np(alloc.dtype)
            out_avals.append(jax.core.ShapedArray(shape, dtype))
            zero_outs.append(np.zeros(shape, dtype))
    n_params = len(in_names)
    n_outs = len(out_avals)
    in_names = in_names + out_names
    if partition_name is not None:
        in_names.append(partition_name)

    def _body(*args):
        operands = list(args)
        if partition_name is not None:
            operands.append(bass2jax.partition_id_tensor())
        outs = _bass_exec_p.bind(
            *operands,
            out_avals=tuple(out_avals),
            in_names=tuple(in_names),
            out_names=tuple(out_names),
            lowering_input_output_aliases=(),
            sim_require_finite=True,
            sim_require_nnan=True,
            nc=nc,
        )
        return tuple(outs)

    devices = jax.devices()[:N_CORES]
    mesh = Mesh(np.asarray(devices), ("core",))
    in_specs = (PartitionSpec("core"),) * (n_params + n_outs)
    out_specs = (PartitionSpec("core"),) * n_outs
    sharded = jax.jit(
        shard_map(
            _body, mesh=mesh, in_specs=in_specs, out_specs=out_specs, check_rep=False
        ),
        keep_unused=True,
    )
    sh = NamedSharding(mesh, PartitionSpec("core"))
    concat_in = [
        jax.device_put(
            np.concatenate(
                [np.asarray(in_maps[c][in_names[i]]) for c in range(N_CORES)], axis=0
            ),
            sh,
        )
        for i in range(n_params)
    ]
    zero_in = [
        jax.device_put(np.concatenate([z] * N_CORES, axis=0), sh) for z in zero_outs
    ]
    # warmup + compile
    out = sharded(*concat_in, *zero_in)
    jax.block_until_ready(out)

    def measure(n):
        jax.block_until_ready(concat_in)
        outs = []
        t0 = _time.perf_counter()
        for _ in range(n):
            outs.append(sharded(*concat_in, *zero_in))
        jax.block_until_ready(outs)
        return _time.perf_counter() - t0

    measure(4)  # warm the dispatch path
    lo, hi = max(8, iters // 4), iters
    t_lo, t_hi = measure(lo), measure(hi)
    marginal = (t_hi - t_lo) / (hi - lo) * 1e9
    per_call = t_hi / hi * 1e9
    print(f"  [time_nc] lo={lo}:{t_lo * 1e3:.1f}ms hi={hi}:{t_hi * 1e3:.1f}ms "
          f"marginal={marginal / 1e3:.1f}us percall={per_call / 1e3:.1f}us")
    return marginal if marginal > 0 else per_call


def kernel(**inputs) -> np.ndarray:
    out, _ = run(inputs)
    return out


# revision 14
# speedup vs baseline: 1.6802x; 1.5384x over previous
"""CrossAttention Trainium2 kernel (8-core SPMD, tensor-parallel over (batch, head-pair)).

Reference computation (full):
    q = x @ Wq; k = ctx @ Wk; v = ctx @ Wv            (per-head split, D=64)
    attn = softmax(q k^T / sqrt(D)) @ v
    out = attn @ Wo + bo

Sharding: core c in [0,8) handles batch b = c // 4 and head-pair hp = c % 4
(heads 2*hp, 2*hp+1 -> 128 "inner" dims). Each core produces a partial output
[Sq, 512] (its two heads' contribution through Wo); the host sums the 4
partials per batch and adds the bias.

Device schedule (per core), built to keep the ScalarE exp stream (the roofline
engine: 33.5M exp elems/core ~ 255us) saturated:
  - scores computed transposed, the two heads' K=64 matmuls packed in PE row
    groups 0/64 (concurrent); attnT accumulation col-packed at partitions
    0-63/64-127 (concurrent); softmax-denominator ones-matmuls col-packed at
    partitions 0/32 (concurrent) -> ~1536 PE cycles per (stile, qchunk) step
    vs 1024-cycle exp.
  - inputs DMA'd in 512-col chunks so the first kt/v tiles (and hence exp)
    start ~5us in; qt for chunk c+1 projected mid-chunk c.
  - chunk boundaries decoupled: at_ps is copied to SBUF right after its last
    accumulation (frees the single PSUM accumulator); denominators alternate
    PSUM partitions 0/32 vs 64/96 between chunks; the reciprocal ->
    DRAM-bounce broadcast -> normalize -> Wo projection chain runs off the
    critical path.
  PSUM: sp 2x[128,1024] (4 banks) + at [128,512] (1) + sm (1) + pp 2x (2).

`_build(repeat=R)` emits the whole body R times in one NEFF; timing two
repeat counts and differencing cancels the ~0.9ms/launch axon dispatch
overhead, isolating true device time per iteration.
"""

import sys

sys.path.insert(0, "/opt/trn_rl_repo")

import numpy as np
import ml_dtypes

BF16 = ml_dtypes.bfloat16

B, SQ, DM = 2, 4096, 512
SKV, DC = 4096, 768
H, D = 8, 64
INNER = H * D  # 512
D2H = 2 * D  # 128, inner dims per core
N_CORES = 8
P = 128
QCHUNK = 512
N_QCHUNK = SQ // QCHUNK  # 8
N_STILE = SKV // P  # 32
KT_Q = DM // P  # 4 k-tiles for the Q projection
KT_KV = DC // P  # 6 k-tiles for the K/V projections
SCALE = float(D) ** -0.5

_COMPILED = None
_COMPILED_R = {}


def _build(repeat=1, no_exp=False, exp_only=False):
    import concourse.bass as bass
    import concourse.tile as tile
    from concourse import bacc, mybir

    fp32 = mybir.dt.float32
    bf16 = mybir.dt.bfloat16
    Exp = mybir.ActivationFunctionType.Exp

    nc = bacc.Bacc(
        "TRN2",
        target_bir_lowering=False,
        debug=False,
        enable_asserts=False,
        num_devices=N_CORES,
    )

    xT = nc.dram_tensor("xT", [DM, SQ], bf16, kind="ExternalInput").ap()
    ctxT = nc.dram_tensor("ctxT", [DC, SKV], bf16, kind="ExternalInput").ap()
    wq = nc.dram_tensor("wq", [DM, D2H], bf16, kind="ExternalInput").ap()
    wk = nc.dram_tensor("wk", [DC, D2H], bf16, kind="ExternalInput").ap()
    wv = nc.dram_tensor("wv", [DC, D2H], bf16, kind="ExternalInput").ap()
    wo = nc.dram_tensor("wo", [D2H, INNER], bf16, kind="ExternalInput").ap()
    out = nc.dram_tensor("out", [SQ, INNER], fp32, kind="ExternalOutput").ap()

    with tile.TileContext(nc) as tc:
        with (
            tc.tile_pool(name="persist", bufs=1) as persist,
            tc.tile_pool(name="pp", bufs=2, space="PSUM") as pp,
            tc.tile_pool(name="spsum", bufs=2, space="PSUM") as spsum,
            tc.tile_pool(name="atpsum", bufs=1, space="PSUM") as atpsum,
            tc.tile_pool(name="smpsum", bufs=1, space="PSUM") as smpsum,
            tc.tile_pool(name="epool", bufs=6) as epool,
            tc.tile_pool(name="apool", bufs=2) as apool,
            tc.tile_pool(name="npool", bufs=2) as npool,
            tc.tile_pool(name="opool", bufs=3) as opool,
            tc.tile_pool(name="dscr", bufs=2, space="DRAM") as dscr,
        ):
            # --- persistent SBUF tensors
            xT_sb = persist.tile([P, KT_Q, SQ], bf16)
            ctxT_sb = persist.tile([P, KT_KV, SKV], bf16)
            wq_sb = persist.tile([P, KT_Q, D2H], bf16)
            wk_sb = persist.tile([P, KT_KV, D2H], bf16)
            wv_sb = persist.tile([P, KT_KV, D2H], bf16)
            wo_sb = persist.tile([P, INNER], bf16)
            qt_sb = persist.tile([P, SQ], bf16)
            kt_sb = persist.tile([P, SQ], bf16)
            v_sb = persist.tile([P, N_STILE * D2H], bf16)
            asc_sb = persist.tile([P, SQ], bf16)  # normalized attnT
            ones_sb = persist.tile([P, 1], bf16)
            junk_sb = persist.tile([P, 8], fp32)

            nc.vector.memset(ones_sb, 1.0)
            # Preload the exp table set during the DMA phase.
            nc.vector.memset(junk_sb, 0.0)
            nc.scalar.activation(out=junk_sb, in_=junk_sb, func=Exp)
            es_const = None
            if no_exp:  # timing-ablation variant: constant weights, no exp
                es_const = persist.tile([P, 2 * QCHUNK], bf16)
                nc.vector.memset(es_const, 0.5)

            def emit_rep(rep):
                r = f"r{rep}_"

                # --- input DMAs, chunked 512 cols so first tiles land early
                def dma_x_chunk(c):
                    cs = slice(c * QCHUNK, (c + 1) * QCHUNK)
                    for t in range(KT_Q):
                        nc.sync.dma_start(
                            out=xT_sb[:, t, cs], in_=xT[t * P : (t + 1) * P, cs]
                        )

                def dma_ctx_chunk(c):
                    cs = slice(c * QCHUNK, (c + 1) * QCHUNK)
                    for t in range(KT_KV):
                        nc.sync.dma_start(
                            out=ctxT_sb[:, t, cs], in_=ctxT[t * P : (t + 1) * P, cs]
                        )

                nc.sync.dma_start(
                    out=wq_sb, in_=wq.rearrange("(t p) m -> p t m", p=P)
                )
                dma_x_chunk(0)
                nc.sync.dma_start(
                    out=wk_sb, in_=wk.rearrange("(t p) m -> p t m", p=P)
                )
                nc.sync.dma_start(
                    out=wv_sb, in_=wv.rearrange("(t p) m -> p t m", p=P)
                )
                dma_ctx_chunk(0)
                nc.sync.dma_start(out=wo_sb, in_=wo)
                for c in range(1, N_QCHUNK):
                    dma_x_chunk(c)
                    dma_ctx_chunk(c)

                # --- projections
                def emit_qt(c):
                    cs = slice(c * QCHUNK, (c + 1) * QCHUNK)
                    ps = pp.tile([P, QCHUNK], fp32, tag="pp", name=f"{r}qtp{c}")
                    for t in range(KT_Q):
                        nc.tensor.matmul(
                            out=ps,
                            lhsT=wq_sb[:, t, :],
                            rhs=xT_sb[:, t, cs],
                            start=(t == 0),
                            stop=(t == KT_Q - 1),
                        )
                    nc.vector.tensor_copy(out=qt_sb[:, cs], in_=ps)

                def emit_kt(c):
                    cs = slice(c * QCHUNK, (c + 1) * QCHUNK)
                    ps = pp.tile([P, QCHUNK], fp32, tag="pp", name=f"{r}ktp{c}")
                    for t in range(KT_KV):
                        nc.tensor.matmul(
                            out=ps,
                            lhsT=wk_sb[:, t, :],
                            rhs=ctxT_sb[:, t, cs],
                            start=(t == 0),
                            stop=(t == KT_KV - 1),
                        )
                    nc.vector.tensor_copy(out=kt_sb[:, cs], in_=ps)

                def emit_v(si):
                    ss = slice(si * P, (si + 1) * P)
                    ps = pp.tile([P, D2H], fp32, tag="pp", name=f"{r}vp{si}")
                    for t in range(KT_KV):
                        nc.tensor.matmul(
                            out=ps,
                            lhsT=ctxT_sb[:, t, ss],
                            rhs=wv_sb[:, t, :],
                            start=(t == 0),
                            stop=(t == KT_KV - 1),
                        )
                    nc.vector.tensor_copy(
                        out=v_sb[:, si * D2H : (si + 1) * D2H], in_=ps
                    )

                def emit_scores_exp(c, si):
                    """scores pair + exp; returns the exp'd weights tile."""
                    cs = slice(c * QCHUNK, (c + 1) * QCHUNK)
                    ss = slice(si * P, (si + 1) * P)
                    sp = spsum.tile(
                        [P, 2 * QCHUNK], fp32, tag="sp", name=f"{r}sp{c}_{si}"
                    )
                    # scores^T, heads row-packed (K=64, PE rows 0/64)
                    nc.tensor.matmul(
                        out=sp[:, 0:QCHUNK],
                        lhsT=kt_sb[0:64, ss],
                        rhs=qt_sb[0:64, cs],
                        start=True,
                        stop=True,
                    )
                    nc.tensor.matmul(
                        out=sp[:, QCHUNK : 2 * QCHUNK],
                        lhsT=kt_sb[64:128, ss],
                        rhs=qt_sb[64:128, cs],
                        start=True,
                        stop=True,
                    )
                    if no_exp:
                        return es_const
                    es = epool.tile(
                        [P, 2 * QCHUNK], bf16, tag="es", name=f"{r}es{c}_{si}"
                    )
                    nc.scalar.activation(out=es, in_=sp, func=Exp, scale=SCALE)
                    return es

                def emit_av_sums(c, si, es, at_ps, sm_ps, p0):
                    """attnV + denominator accumulation for a step.

                    Emitted AFTER the NEXT step's scores matmuls (software
                    pipelining): these matmuls wait on exp(si), and on the
                    in-order PE queue they must not block scores(si+1), or
                    the ScalarE exp stream (the bottleneck) runs at
                    exp+matmul serial rate instead of back-to-back.
                    """
                    # attnT accumulation, heads col-packed (PE cols 0/64)
                    vs = si * D2H
                    nc.tensor.matmul(
                        out=at_ps[0:64, :],
                        lhsT=v_sb[:, vs : vs + 64],
                        rhs=es[:, 0:QCHUNK],
                        start=(si == 0),
                        stop=(si == N_STILE - 1),
                    )
                    nc.tensor.matmul(
                        out=at_ps[64:128, :],
                        lhsT=v_sb[:, vs + 64 : vs + 128],
                        rhs=es[:, QCHUNK : 2 * QCHUNK],
                        start=(si == 0),
                        stop=(si == N_STILE - 1),
                        # sim's psum group-tracking is partition-blind; HW
                        # tracks per-element has_written, so col-packed
                        # groups are safe
                        skip_group_check=True,
                    )
                    # softmax denominators, col-packed at partitions p0 / p0+32
                    # (p0 alternates 0/64 between chunks: no inter-chunk hazard)
                    nc.tensor.matmul(
                        out=sm_ps[p0 : p0 + 1, :],
                        lhsT=ones_sb,
                        rhs=es[:, 0:QCHUNK],
                        start=(si == 0),
                        stop=(si == N_STILE - 1),
                        tile_position=(0, p0),
                    )
                    nc.tensor.matmul(
                        out=sm_ps[p0 + 32 : p0 + 33, :],
                        lhsT=ones_sb,
                        rhs=es[:, QCHUNK : 2 * QCHUNK],
                        start=(si == 0),
                        stop=(si == N_STILE - 1),
                        tile_position=(0, p0 + 32),
                        skip_group_check=True,
                    )

                def finish_chunk(c, at_ps, sm_ps, p0):
                    """Free the accumulators, then normalize + project."""
                    cs = slice(c * QCHUNK, (c + 1) * QCHUNK)
                    araw = apool.tile(
                        [P, QCHUNK], bf16, tag="araw", name=f"{r}ar{c}"
                    )
                    nc.vector.tensor_copy(out=araw, in_=at_ps)
                    rec_sb = npool.tile(
                        [P, QCHUNK], fp32, tag="rec", name=f"{r}rc{c}"
                    )
                    nc.vector.reciprocal(
                        out=rec_sb[0:1, :], in_=sm_ps[p0 : p0 + 1, :]
                    )
                    nc.vector.reciprocal(
                        out=rec_sb[32:33, :], in_=sm_ps[p0 + 32 : p0 + 33, :]
                    )
                    dt = dscr.tile([2, QCHUNK], fp32, tag="dt", name=f"{r}dt{c}")
                    nc.sync.dma_start(out=dt[0:1, :], in_=rec_sb[0:1, :])
                    nc.sync.dma_start(out=dt[1:2, :], in_=rec_sb[32:33, :])
                    rep_sb = npool.tile(
                        [P, QCHUNK], fp32, tag="rep", name=f"{r}rp{c}"
                    )
                    bcast0 = bass.AP(
                        tensor=dt.tensor, offset=dt.offset, ap=[[0, 64], [1, QCHUNK]]
                    )
                    bcast1 = bass.AP(
                        tensor=dt.tensor,
                        offset=dt.offset + QCHUNK,
                        ap=[[0, 64], [1, QCHUNK]],
                    )
                    nc.sync.dma_start(out=rep_sb[0:64, :], in_=bcast0)
                    nc.sync.dma_start(out=rep_sb[64:128, :], in_=bcast1)
                    nc.vector.tensor_mul(asc_sb[:, cs], araw, rep_sb)
                    for qt in range(QCHUNK // P):
                        r0 = c * QCHUNK + qt * P
                        po = pp.tile(
                            [P, INNER], fp32, tag="pp", name=f"{r}po{c}_{qt}"
                        )
                        nc.tensor.matmul(
                            out=po,
                            lhsT=asc_sb[:, r0 : r0 + P],
                            rhs=wo_sb,
                            start=True,
                            stop=True,
                        )
                        ob = opool.tile(
                            [P, INNER], fp32, tag="ob", name=f"{r}ob{c}_{qt}"
                        )
                        nc.vector.tensor_copy(out=ob, in_=po)
                        nc.sync.dma_start(out=out[r0 : r0 + P, :], in_=ob)

                # --- software-pipelined step scheduler: attnV/sums of step i
                # are emitted after scores of step i+1 (see emit_av_sums).
                pend = [None]

                def schedule_step(c, si, at_ps, sm_ps, p0):
                    es = emit_scores_exp(c, si)
                    if exp_only:
                        return
                    prev, pend[0] = pend[0], None
                    if prev is not None:
                        emit_av_sums(*prev)
                    pend[0] = (c, si, es, at_ps, sm_ps, p0)

                # --- phase A: chunk-0 attention interleaved with kt/v production
                emit_qt(0)
                at_ps = atpsum.tile([P, QCHUNK], fp32, tag="at", name=f"{r}at0")
                sm_ps = smpsum.tile([P, QCHUNK], fp32, tag="sm", name=f"{r}sm0")
                for ck in range(N_QCHUNK):
                    emit_kt(ck)
                    for si in range(4 * ck, 4 * ck + 4):
                        emit_v(si)
                        schedule_step(0, si, at_ps, sm_ps, 0)
                emit_qt(1)
                prev_finish = (0, at_ps, sm_ps, 0)

                # --- remaining q-chunks; the previous chunk's normalization +
                # Wo projection are emitted after this chunk's first scores
                # (its last attnV flushes there); qt for the next chunk is
                # projected mid-chunk so its copy lands before the boundary.
                for c in range(1, N_QCHUNK):
                    p0 = 64 * (c % 2)
                    at_ps = atpsum.tile(
                        [P, QCHUNK], fp32, tag="at", name=f"{r}at{c}"
                    )
                    sm_ps = smpsum.tile(
                        [P, QCHUNK], fp32, tag="sm", name=f"{r}sm{c}"
                    )
                    for si in range(N_STILE):
                        schedule_step(c, si, at_ps, sm_ps, p0)
                        if si == 0 and not exp_only:
                            finish_chunk(*prev_finish)
                        if si == 16 and c < N_QCHUNK - 1:
                            emit_qt(c + 1)
                    prev_finish = (c, at_ps, sm_ps, p0)

                if pend[0] is not None:
                    emit_av_sums(*pend[0])
                    pend[0] = None
                if not exp_only:
                    finish_chunk(*prev_finish)

            for rep in range(repeat):
                emit_rep(rep)

    nc.compile()
    return nc


def _get_compiled():
    global _COMPILED
    if _COMPILED is None:
        _COMPILED = _build()
    return _COMPILED


def _get_compiled_r(repeat):
    if repeat == 1:
        return _get_compiled()
    if repeat not in _COMPILED_R:
        _COMPILED_R[repeat] = _build(repeat)
    return _COMPILED_R[repeat]


def _make_in_maps(x, context, Wq, Wk, Wv, Wo):
    xT = [np.ascontiguousarray(x[b].T).astype(BF16) for b in range(B)]
    ctxT = [np.ascontiguousarray(context[b].T).astype(BF16) for b in range(B)]
    wq16, wk16 = Wq.astype(BF16), Wk.astype(BF16)
    wv16, wo16 = Wv.astype(BF16), Wo.astype(BF16)
    in_maps = []
    for core in range(N_CORES):
        b, hp = core // 4, core % 4
        js = slice(hp * D2H, (hp + 1) * D2H)
        in_maps.append(
            {
                "xT": xT[b],
                "ctxT": ctxT[b],
                "wq": np.ascontiguousarray(wq16[:, js]),
                "wk": np.ascontiguousarray(wk16[:, js]),
                "wv": np.ascontiguousarray(wv16[:, js]),
                "wo": np.ascontiguousarray(wo16[js, :]),
            }
        )
    return in_maps


def run(inputs, **kw):
    """Run on hardware; returns (full_output, results list)."""
    from concourse import bass2jax

    nc = _get_compiled()
    in_maps = _make_in_maps(
        inputs["x"], inputs["context"], inputs["Wq"], inputs["Wk"],
        inputs["Wv"], inputs["Wo"],
    )
    results = bass2jax.run_bass_via_pjrt(nc, in_maps, n_cores=N_CORES)
    bo = inputs["bo"]
    out = np.empty((B, SQ, INNER), np.float32)
    for b in range(B):
        acc = results[4 * b]["out"].astype(np.float32)
        for hp in range(1, 4):
            acc = acc + results[4 * b + hp]["out"]
        out[b] = acc + np.asarray(bo, np.float32)[None, :]
    return out, results


def time_exec(inputs, iters=48):
    """True device time per kernel iteration, in ns.

    Per-XLA-call dispatch through the axon tunnel costs ~0.7-0.9ms and masks
    device time entirely (a trivial kernel's per-call marginal measures the
    same as this kernel's). So we compile two NEFFs that run the body R=4 and
    R=8 times and difference the per-call marginals: (m8 - m4) / 4 cancels
    the dispatch constant exactly, leaving pure device execution time.
    """
    in_maps = _make_in_maps(
        inputs["x"], inputs["context"], inputs["Wq"], inputs["Wk"],
        inputs["Wv"], inputs["Wo"],
    )
    m4 = time_nc(_get_compiled_r(4), in_maps, iters=iters)
    m8 = time_nc(_get_compiled_r(8), in_maps, iters=iters)
    exec_ns = (m8 - m4) / 4.0
    print(f"  [time_exec] m4={m4 / 1e3:.1f}us m8={m8 / 1e3:.1f}us "
          f"-> exec {exec_ns / 1e3:.1f}us")
    return exec_ns


def time_nc(nc, in_maps, iters=48):
    """Amortized wall time per XLA call (includes dispatch overhead), in ns."""
    import time as _time

    import jax
    from jax.sharding import Mesh, NamedSharding, PartitionSpec
    from concourse import bass2jax, mybir
    from concourse.bass2jax import _bass_exec_p, install_neuronx_cc_hook

    try:
        from jax.experimental.shard_map import shard_map
    except ImportError:
        from jax.shard_map import shard_map

    install_neuronx_cc_hook()

    partition_name = nc.partition_id_tensor.name if nc.partition_id_tensor else None
    in_names, out_names, out_avals, zero_outs = [], [], [], []
    for alloc in nc.m.functions[0].allocations:
        if not isinstance(alloc, mybir.MemoryLocationSet):
            continue
        name = alloc.memorylocations[0].name
        if alloc.kind == "ExternalInput":
            if name != partition_name:
                in_names.append(name)
        elif alloc.kind == "ExternalOutput":
            out_names.append(name)
            shape = tuple(alloc.tensor_shape)
            dtype = mybir.dt.np(alloc.dtype)
            out_avals.append(jax.core.ShapedArray(shape, dtype))
            zero_outs.append(np.zeros(shape, dtype))
    n_params = len(in_names)
    n_outs = len(out_avals)
    in_names = in_names + out_names
    if partition_name is not None:
        in_names.append(partition_name)

    def _body(*args):
        operands = list(args)
        if partition_name is not None:
            operands.append(bass2jax.partition_id_tensor())
        outs = _bass_exec_p.bind(
            *operands,
            out_avals=tuple(out_avals),
            in_names=tuple(in_names),
            out_names=tuple(out_names),
            lowering_input_output_aliases=(),
            sim_require_finite=True,
            sim_require_nnan=True,
            nc=nc,
        )
        return tuple(outs)

    devices = jax.devices()[:N_CORES]
    mesh = Mesh(np.asarray(devices), ("core",))
    in_specs = (PartitionSpec("core"),) * (n_params + n_outs)
    out_specs = (PartitionSpec("core"),) * n_outs
    sharded = jax.jit(
        shard_map(
            _body, mesh=mesh, in_specs=in_specs, out_specs=out_specs, check_rep=False
        ),
        keep_unused=True,
    )
    sh = NamedSharding(mesh, PartitionSpec("core"))
    concat_in = [
        jax.device_put(
            np.concatenate(
                [np.asarray(in_maps[c][in_names[i]]) for c in range(N_CORES)], axis=0
            ),
            sh,
        )
        for i in range(n_params)
    ]
    zero_in = [
        jax.device_put(np.concatenate([z] * N_CORES, axis=0), sh) for z in zero_outs
    ]
    # warmup + compile
    out = sharded(*concat_in, *zero_in)
    jax.block_until_ready(out)

    def measure(n):
        jax.block_until_ready(concat_in)
        outs = []
        t0 = _time.perf_counter()
        for _ in range(n):
            outs.append(sharded(*concat_in, *zero_in))
        jax.block_until_ready(outs)
        return _time.perf_counter() - t0

    measure(4)  # warm the dispatch path
    lo, hi = max(8, iters // 4), iters
    t_lo, t_hi = measure(lo), measure(hi)
    marginal = (t_hi - t_lo) / (hi - lo) * 1e9
    per_call = t_hi / hi * 1e9
    print(f"  [time_nc] lo={lo}:{t_lo * 1e3:.1f}ms hi={hi}:{t_hi * 1e3:.1f}ms "
          f"marginal={marginal / 1e3:.1f}us percall={per_call / 1e3:.1f}us")
    return marginal if marginal > 0 else per_call


def kernel(**inputs) -> np.ndarray:
    out, _ = run(inputs)
    return out
